# revision 29
# baseline (speedup 1.0000x reference)
"""Trainium2 Bass kernel for 16-head MHA (d_model=1024, batch 4, seq 2048).

Sharding: batch (4) x head-group (2) across 8 NeuronCores. Each core computes
one batch sample's attention for 8 of the 16 heads plus its partial output
projection; the host sums the two partial outputs per sample and adds the
bias terms.

Per-core dataflow (all matmul contractions run on the partition axis):
  q^T/k^T = WqT'.T @ x^T   (fp16, out [o, t] with heads on partitions)
  v       = x^T.T @ WvT    (fp16 matmul, bf16 store, [t, o] with a ones column
                            per head for fused softmax rowsums)
  E^T     = exp(kT_h.T @ qT_h)  (scores f32 in PSUM, exp on ACT, bf16 store;
                                 no max-subtraction: reference applies no
                                 1/sqrt(d) scaling and scores stay < ~50)
  att_h   = (V_h|1).T @ E^T_h   -> rows 0:64 raw attention, row 64 rowsum
  att^T   = att_h * recip(rowsum)
  y^T     = WoT'.T @ att^T      (bf16 out, host sums the two partials in f32)

The run is ACT(exp)-limited in steady state: 256 exps of [128,1024] at
(1024+352)/1.2 ns each = 293us. The schedule keeps ACT saturated:

- The two heads of a pair run their score matmuls CONCURRENTLY as row-tiled
  MMs (K=64 each, row groups 0/64) into one fused [128, 2*512] PSUM tile, so
  one exp covers the pair and s1 PE time halves vs per-head issue.
- Attention runs as ONE flat software pipeline over 16 blocks of (head-pair x
  query-quarter) x 16 key chunks; att@V trails the exp by 2 chunks and flows
  across block boundaries with no drain.
- Projections are split into ~2-matmul microtasks and dripped into the kc
  loops (1 pull/kc) in data-arrival order; a filler feeding block N is always
  fully emitted before block N's first reader (Tile deps only point backward).
- DMA is descriptor-row-bound (~25ns/row), so the hot transfers use fat rows:
  the host pre-packs weights as [128, ic*512] (8KB rows) and xv as eight
  512KB t-chunk-major blocks (4KB rows, each holding all 8 contraction slices
  for 2 key chunks).  Queues: sync = wv + xv blocks (v chase), scalar = wk +
  xk rows (k chase), gpsimd = wq + biases + xq rows + Wo.

fp16 is used for the whole q/k/scores path: bf16's 8-bit mantissa gives score
errors ~0.05 which exp() amplifies to ~2e-2 output error; fp16 keeps it ~3e-3.
"""

from collections import deque
from contextlib import ExitStack

import numpy as np

import concourse.bacc as bacc
import concourse.mybir as mybir
import concourse.tile as tile
from concourse.bass_utils import run_bass_kernel_spmd

F32 = mybir.dt.float32
F16 = mybir.dt.float16
BF16 = mybir.dt.bfloat16

D = 1024          # d_model
HD = 64           # head dim
NH_CORE = 8       # heads per core
OC = NH_CORE * HD # per-core q/k/v output dims (512)
N_CORES = 8
NI = D // 128     # contraction chunks for projections
NOC = OC // 128   # o-chunks (head pairs)
NDC = D // 128    # output-dim chunks for the final projection
VG = 256          # xv host block width (2 key chunks)


def build_kernel(S=2048):
    nc = bacc.Bacc("TRN2", target_bir_lowering=False, debug=False)

    NVG = S // VG             # 8 xv blocks

    xq_d = nc.dram_tensor("xqT", (D, S), F16, kind="ExternalInput")
    xk_d = nc.dram_tensor("xkT", (D, S), F16, kind="ExternalInput")
    xv_d = nc.dram_tensor("xvB", (NVG, 128, NI * VG), F16, kind="ExternalInput")
    wq_d = nc.dram_tensor("WqP", (128, NI * OC), F16, kind="ExternalInput")
    wk_d = nc.dram_tensor("WkP", (128, NI * OC), F16, kind="ExternalInput")
    wv_d = nc.dram_tensor("WvP", (128, NI * OC), F16, kind="ExternalInput")
    wo_d = nc.dram_tensor("WoP", (128, NOC * D), F16, kind="ExternalInput")
    bq_d = nc.dram_tensor("bq", (OC,), F32, kind="ExternalInput")
    bk_d = nc.dram_tensor("bk", (OC,), F32, kind="ExternalInput")
    y_d = nc.dram_tensor("yT", (D, S), BF16, kind="ExternalOutput")

    NKC = S // 128            # key chunks (16)
    QQ = 512                  # query-quarter size (s1 moving width per head)
    NQQ = S // QQ             # 4

    Exp = mybir.ActivationFunctionType.Exp
    Mult = mybir.AluOpType.mult

    with tile.TileContext(nc) as tc, ExitStack() as ctx:
        wpool = ctx.enter_context(tc.tile_pool(name="w", bufs=1))
        xpool = ctx.enter_context(tc.tile_pool(name="x", bufs=1))
        spool = ctx.enter_context(tc.tile_pool(name="seq", bufs=1))
        epool = ctx.enter_context(tc.tile_pool(name="e", bufs=3))
        evpool = ctx.enter_context(tc.tile_pool(name="ev", bufs=2))
        npool = ctx.enter_context(tc.tile_pool(name="nrm", bufs=2))
        pjpool = ctx.enter_context(tc.tile_pool(name="pj", bufs=2, space="PSUM"))
        s1pool = ctx.enter_context(tc.tile_pool(name="s1", bufs=2, space="PSUM"))
        accpool = ctx.enter_context(tc.tile_pool(name="acc", bufs=2, space="PSUM"))

        # ---- resident weights / biases / x slabs ----
        wq_sb = wpool.tile([128, NI, OC], F16, tag="wq")
        wk_sb = wpool.tile([128, NI, OC], F16, tag="wk")
        wv_sb = wpool.tile([128, NI, OC], F16, tag="wv")
        wo_sb = wpool.tile([128, NOC, D], F16, tag="wo")
        bq_sb = wpool.tile([128, NOC], F32, tag="bq")
        bk_sb = wpool.tile([128, NOC], F32, tag="bk")

        xq_sb = xpool.tile([128, NI, S], F16, tag="xq")
        xk_sb = xpool.tile([128, NI, S], F16, tag="xk")
        xvg_sb = [xpool.tile([128, NI, VG], F16, tag="xvg", bufs=4,
                             name=f"xvg{g}") for g in range(NVG)]

        # sync HW queue: wv (8KB rows) then xv blocks (4KB rows) -> v chase
        nc.sync.dma_start(out=wv_sb,
                          in_=wv_d.ap().rearrange("p (ic o) -> p ic o", ic=NI))
        for g in range(NVG):
            nc.sync.dma_start(out=xvg_sb[g],
                              in_=xv_d.ap()[g].rearrange("p (ic t) -> p ic t", ic=NI))
        # scalar HW queue: wk + xk full rows (4KB) -> k chase
        nc.scalar.dma_start(out=wk_sb,
                            in_=wk_d.ap().rearrange("p (ic o) -> p ic o", ic=NI))
        for ic in range(NI):
            nc.scalar.dma_start(out=xk_sb[:, ic, :],
                                in_=xk_d.ap()[ic * 128:(ic + 1) * 128, :])
        # gpsimd SW queue: wq, biases, xq full rows, wo (needed latest)
        nc.gpsimd.dma_start(out=wq_sb,
                            in_=wq_d.ap().rearrange("p (ic o) -> p ic o", ic=NI))
        nc.gpsimd.dma_start(out=bq_sb, in_=bq_d.ap().rearrange("(c p) -> p c", p=128))
        nc.gpsimd.dma_start(out=bk_sb, in_=bk_d.ap().rearrange("(c p) -> p c", p=128))
        for ic in range(NI):
            nc.gpsimd.dma_start(out=xq_sb[:, ic, :],
                                in_=xq_d.ap()[ic * 128:(ic + 1) * 128, :])
        nc.gpsimd.dma_start(out=wo_sb,
                            in_=wo_d.ap().rearrange("p (oc d) -> p oc d", oc=NOC))

        # ---- per-sequence slabs ----
        v_sb = spool.tile([128, NKC, NH_CORE * (HD + 1)], BF16, tag="v")
        qT_sb = spool.tile([128, NOC, S], F16, tag="qT")
        kT_sb = spool.tile([128, NOC, S], F16, tag="kT")
        att_sb = spool.tile([128, NOC, S], BF16, tag="att")

        # ---- microtask unit factories (units are atomic: each holds its pj
        # ---- ring slots until its trailing DVE op) ----
        def v_sub(g):
            """v projection for one xv block (2 key chunks): 16 MMs + copy."""
            state = {}
            def mk_mm(ic):
                def f():
                    if "a" not in state:
                        state["a"] = pjpool.tile([128, OC], F32, tag="pj",
                                                 name=f"vpsa{g}")
                        state["b"] = pjpool.tile([128, OC], F32, tag="pj",
                                                 name=f"vpsb{g}")
                    for j, key in enumerate(("a", "b")):
                        nc.tensor.matmul(
                            state[key][:, :],
                            xvg_sb[g][:, ic, j * 128:(j + 1) * 128],
                            wv_sb[:, ic, :],
                            start=(ic == 0), stop=(ic == NI - 1),
                        )
                return f
            def fin():
                for j, key in enumerate(("a", "b")):
                    vv = v_sb[:, 2 * g + j, :].rearrange("p (h c) -> p h c",
                                                         h=NH_CORE)
                    nc.vector.tensor_copy(
                        out=vv[:, :, 0:HD],
                        in_=state[key][:, :].rearrange("p (h c) -> p h c", c=HD),
                    )
                    nc.vector.memset(vv[:, :, HD:HD + 1], 1.0)
            return [mk_mm(ic) for ic in range(NI)] + [fin]

        def proj_pair(w_sb, x_sb, b_sb, dst, hp, tqa, tqb, label):
            """q/k projection for (head-pair, two 512-col chunks): each
            stationary w[ic] feeds both moving chunks."""
            state = {}
            def mk_mm(ic):
                def f():
                    if "a" not in state:
                        state["a"] = pjpool.tile([128, QQ], F32, tag="pj",
                                                 name=f"pj{label}{hp}_{tqa}")
                        state["b"] = pjpool.tile([128, QQ], F32, tag="pj",
                                                 name=f"pj{label}{hp}_{tqb}")
                    for key, tq in (("a", tqa), ("b", tqb)):
                        nc.tensor.matmul(
                            state[key][:, :],
                            w_sb[:, ic, hp * 128:(hp + 1) * 128],
                            x_sb[:, ic, tq * QQ:(tq + 1) * QQ],
                            start=(ic == 0), stop=(ic == NI - 1),
                        )
                return f
            def bias():
                for key, tq in (("a", tqa), ("b", tqb)):
                    nc.vector.tensor_scalar_add(
                        out=dst[:, hp, tq * QQ:(tq + 1) * QQ],
                        in0=state[key][:, :],
                        scalar1=b_sb[:, hp:hp + 1],
                    )
            return [mk_mm(ic) for ic in range(NI)] + [bias]

        def y_unit(qq, dc):
            """output projection for one (128 d rows, 512 queries):
            4 accum matmuls as 2 microtasks + bf16 store."""
            state = {}
            def mk_mm(ocp):
                def f():
                    if "pj" not in state:
                        state["pj"] = pjpool.tile([128, QQ], F32, tag="pj",
                                                  name=f"yps{qq}_{dc}")
                    for oc in (2 * ocp, 2 * ocp + 1):
                        nc.tensor.matmul(
                            state["pj"][:, :],
                            wo_sb[:, oc, dc * 128:(dc + 1) * 128],
                            att_sb[:, oc, qq * QQ:(qq + 1) * QQ],
                            start=(oc == 0), stop=(oc == NOC - 1),
                        )
                return f
            def store():
                y_sb = evpool.tile([128, QQ], BF16, tag="yev",
                                   name=f"yev{qq}_{dc}")
                nc.vector.tensor_copy(out=y_sb[:, :], in_=state["pj"][:, :])
                nc.sync.dma_start(
                    out=y_d.ap()[dc * 128:(dc + 1) * 128,
                                 qq * QQ:(qq + 1) * QQ],
                    in_=y_sb[:, :],
                )
            return [mk_mm(0), mk_mm(1), store]

        # ---- filler schedule ----
        fillers = deque()

        def pull(n=1):
            for _ in range(n):
                if fillers:
                    fillers.popleft()()

        def run_unit(u):
            for f in u:
                f()

        # ---- prologue: v proj chasing the xv blocks, hp0 k/q interleaved
        # ---- in arrival order.  Units are atomic (pj slot discipline).
        v_sub_units = [v_sub(g) for g in range(NVG)]
        run_unit(v_sub_units[0])
        run_unit(v_sub_units[1])
        run_unit(v_sub_units[2])
        run_unit(proj_pair(wk_sb, xk_sb, bk_sb, kT_sb, 0, 0, 1, "k"))
        run_unit(v_sub_units[3])
        run_unit(proj_pair(wk_sb, xk_sb, bk_sb, kT_sb, 0, 2, 3, "k"))
        run_unit(v_sub_units[4])
        run_unit(v_sub_units[5])
        run_unit(proj_pair(wq_sb, xq_sb, bq_sb, qT_sb, 0, 0, 1, "q"))
        run_unit(v_sub_units[6])
        run_unit(v_sub_units[7])

        # ---- filler population in need-order ----
        # Block (hp,0) starts at pull 32*hp; each pair's k+q(0,1) units (27
        # microtasks) sit within the preceding 32 pulls.
        for hp in (1, 2, 3):
            fillers.extend(proj_pair(wk_sb, xk_sb, bk_sb, kT_sb, hp, 0, 1, "k"))
            fillers.extend(proj_pair(wk_sb, xk_sb, bk_sb, kT_sb, hp, 2, 3, "k"))
            fillers.extend(proj_pair(wq_sb, xq_sb, bq_sb, qT_sb, hp, 0, 1, "q"))
        # hp0's remaining q quarters are not needed until block 8 (pull 128)
        fillers.extend(proj_pair(wq_sb, xq_sb, bq_sb, qT_sb, 0, 2, 3, "q"))

        later_q = []
        for hp in (1, 2, 3):
            later_q.append(proj_pair(wq_sb, xq_sb, bq_sb, qT_sb, hp, 2, 3, "q"))

        # block order: quarters 0/1 paired per head-pair (gives the k/q
        # prefetch two blocks of lead per pair), then quarters 2/3 swept
        # by-quarter so y(qq2) can run as filler and only y(qq3) tails
        order = ([(hp, qq) for hp in range(NOC) for qq in (0, 1)]
                 + [(hp, 2) for hp in range(NOC)]
                 + [(hp, 3) for hp in range(NOC)])

        # ---- flat attention pipeline ----
        pend = deque()
        accs_by_bi = {}
        qq_done = {qq: 0 for qq in range(NQQ)}

        def finalize(bi, hp, qq):
            # copy accs to SBUF first: frees the acc PSUM ring slots early so
            # the next block's s2 never stalls on them
            qoff = qq * QQ
            accs = accs_by_bi.pop(bi)
            asbs = []
            for hl in range(2):
                asb = npool.tile([65, QQ], F32, tag="accsb", bufs=2,
                                 name=f"asb{bi}_{hl}")
                nc.vector.tensor_copy(out=asb[:, :], in_=accs[hl][0:65, :])
                asbs.append(asb)
            for hl in range(2):
                off = hl * 64
                asb = asbs[hl]
                rt = npool.tile([1, QQ], F32, tag="rtmp", bufs=2,
                                name=f"rt{bi}_{hl}")
                nc.vector.tensor_copy(out=rt[:, :], in_=asb[64:65, :])
                nc.vector.reciprocal_approx_fast(out=rt[:, :], in_=rt[:, :])
                bc = npool.tile([64, QQ], F32, tag="bcast", bufs=2,
                                name=f"bc{bi}_{hl}")
                nc.gpsimd.partition_broadcast(out_ap=bc[:, :], in_ap=rt[:, :])
                nc.vector.tensor_tensor(
                    out=att_sb[off:off + 64, hp, qoff:qoff + QQ],
                    in0=asb[0:64, :],
                    in1=bc[:, :],
                    op=Mult,
                )
            qq_done[qq] += 1
            if qq_done[qq] == NOC and qq < 3:
                for dc in range(NDC):
                    fillers.extend(y_unit(qq, dc))
            # qq3's y is emitted in the tail (acc ring, hp3-oc last) so most
            # of it overlaps the final block's normalize

        def s2_pop():
            bi2, hp2, qq2, e2, kc2 = pend.popleft()
            if kc2 == 0:
                accs_by_bi[bi2] = [
                    accpool.tile([128, QQ], F32, tag="acc", name=f"acc{bi2}_{hl}")
                    for hl in range(2)
                ]
            accs = accs_by_bi[bi2]
            for hl in range(2):
                h = 2 * hp2 + hl
                nc.tensor.matmul(
                    accs[hl][0:65, :],
                    v_sb[:, kc2, h * (HD + 1):(h + 1) * (HD + 1)],
                    e2[:, hl * QQ:(hl + 1) * QQ],
                    start=(kc2 == 0), stop=(kc2 == NKC - 1),
                )
            if kc2 == NKC - 1:
                finalize(bi2, hp2, qq2)

        for bi, (hp, qq) in enumerate(order):
            if bi == 4:
                for u in later_q:
                    fillers.extend(u)
            qoff = qq * QQ
            for kc in range(NKC):
                s1 = s1pool.tile([128, 2 * QQ], F32, tag="s1",
                                 name=f"s1_{bi}_{kc}")
                # the two heads' score MMs land on row groups 0/64 and
                # stream concurrently; one exp covers the fused tile
                for hl in range(2):
                    off = hl * 64
                    nc.tensor.matmul(
                        s1[:, hl * QQ:(hl + 1) * QQ],
                        kT_sb[off:off + 64, hp, kc * 128:(kc + 1) * 128],
                        qT_sb[off:off + 64, hp, qoff:qoff + QQ],
                        start=True, stop=True,
                    )
                e = epool.tile([128, 2 * QQ], BF16, tag="e", name=f"e{bi}_{kc}")
                nc.scalar.activation(out=e[:, :], in_=s1[:, :], func=Exp)
                pend.append((bi, hp, qq, e, kc))
                if len(pend) > 2:
                    s2_pop()
                pull(1)
        while pend:
            s2_pop()

        # tail: y(qq3) through the freed acc PSUM ring (2-deep pipelining);
        # oc0-2 matmuls run during the final normalize (their att quarters
        # are long ready), only the hp3-dependent oc3 waits for it
        for dc in range(NDC):
            state = {}
            yp = accpool.tile([128, QQ], F32, tag="acc", name=f"yt{dc}")
            for ocs in ((0, 1), (2,), (3,)):
                for oc in ocs:
                    nc.tensor.matmul(
                        yp[:, :],
                        wo_sb[:, oc, dc * 128:(dc + 1) * 128],
                        att_sb[:, oc, 3 * QQ:4 * QQ],
                        start=(oc == 0), stop=(oc == NOC - 1),
                    )
            y_sb = evpool.tile([128, QQ], BF16, tag="yev", name=f"yevt{dc}")
            nc.vector.tensor_copy(out=y_sb[:, :], in_=yp[:, :])
            nc.sync.dma_start(
                out=y_d.ap()[dc * 128:(dc + 1) * 128, 3 * QQ:4 * QQ],
                in_=y_sb[:, :],
            )
        while fillers:
            fillers.popleft()()

    nc.compile()
    return nc


def make_in_maps(query, key, value, Wq, bq, Wk, bk, Wv, bv, Wo, bo):
    """Shard + lay out full inputs for the 8 cores: core = 2*n + g.

    Weights are host-packed to [128, ic*512] (8KB DMA rows) and xv to
    t-chunk-major 512KB blocks (4KB rows) — DMA is descriptor-row-bound."""
    f16 = np.float16
    N, S, _ = query.shape
    NVG = S // VG

    def pack_w(WT):  # [D, oc] -> [128, ic*oc]
        ni, oc = WT.shape[0] // 128, WT.shape[1]
        return np.ascontiguousarray(
            WT.reshape(ni, 128, oc).transpose(1, 0, 2).reshape(128, ni * oc)
        ).astype(f16)

    per_g = {}
    for g in range(2):
        osl = slice(g * OC, (g + 1) * OC)
        per_g[g] = dict(
            WqP=pack_w(Wq[osl, :].T),
            WkP=pack_w(Wk[osl, :].T),
            WvP=pack_w(Wv[osl, :].T),
            WoP=pack_w(Wo[:, osl].T),
            bq=np.ascontiguousarray(bq[osl]).astype(np.float32),
            bk=np.ascontiguousarray(bk[osl]).astype(np.float32),
        )
    in_maps = []
    for n in range(N):
        xqT = np.ascontiguousarray(query[n].T).astype(f16)
        xkT = np.ascontiguousarray(key[n].T).astype(f16)
        xvT = value[n].T  # [D, S]
        xvB = np.ascontiguousarray(
            xvT.reshape(NI, 128, NVG, VG).transpose(2, 1, 0, 3)
               .reshape(NVG, 128, NI * VG)
        ).astype(f16)
        for g in range(2):
            m = dict(xqT=xqT, xkT=xkT, xvB=xvB)
            m.update(per_g[g])
            in_maps.append(m)
    return in_maps


_BUILT = None


def _get_built():
    global _BUILT
    if _BUILT is None:
        _BUILT = build_kernel(2048)
    return _BUILT


def kernel(query, key, value, Wq, bq, Wk, bk, Wv, bv, Wo, bo, _results=None):
    query = np.asarray(query, np.float32)
    key = np.asarray(key, np.float32)
    value = np.asarray(value, np.float32)
    Wq, bq = np.asarray(Wq, np.float32), np.asarray(bq, np.float32)
    Wk, bk = np.asarray(Wk, np.float32), np.asarray(bk, np.float32)
    Wv, bv = np.asarray(Wv, np.float32), np.asarray(bv, np.float32)
    Wo, bo = np.asarray(Wo, np.float32), np.asarray(bo, np.float32)

    N, S, _ = query.shape
    if _results is None:
        nc = _get_built()
        in_maps = make_in_maps(query, key, value, Wq, bq, Wk, bk, Wv, bv, Wo, bo)
        res = run_bass_kernel_spmd(nc, in_maps, list(range(N_CORES)))
        _results = res.results

    const = bv @ Wo.T + bo  # host-folded bias terms
    out = np.empty((N, S, D), np.float32)
    for n in range(N):
        yT = (_results[2 * n]["yT"].astype(np.float32)
              + _results[2 * n + 1]["yT"].astype(np.float32))
        out[n] = yT.T + const
    return out


# revision 30
# speedup vs baseline: 1.0180x; 1.0180x over previous
"""Trainium2 Bass kernel for 16-head MHA (d_model=1024, batch 4, seq 2048).

Sharding: batch (4) x head-group (2) across 8 NeuronCores. Each core computes
one batch sample's attention for 8 of the 16 heads plus its partial output
projection; the host sums the two partial outputs per sample and adds the
bias terms.

Per-core dataflow (all matmul contractions run on the partition axis):
  q^T/k^T = WqT'.T @ x^T   (fp16, out [o, t] with heads on partitions)
  v       = x^T.T @ WvT    (fp16 matmul, bf16 store, [t, o] with a ones column
                            per head for fused softmax rowsums)
  E^T     = exp(kT_h.T @ qT_h)  (scores f32 in PSUM, exp on ACT, bf16 store;
                                 no max-subtraction: reference applies no
                                 1/sqrt(d) scaling and scores stay < ~50)
  att_h   = (V_h|1).T @ E^T_h   -> rows 0:64 raw attention, row 64 rowsum
  att^T   = att_h * recip(rowsum)
  y^T     = WoT'.T @ att^T      (bf16 out, host sums the two partials in f32)

The run is ACT(exp)-limited in steady state: 256 exps of [128,1024] at
(1024+352)/1.2 ns each = 293us. The schedule keeps ACT saturated:

- The two heads of a pair run their score matmuls CONCURRENTLY as row-tiled
  MMs (K=64 each, row groups 0/64) into one fused [128, 2*512] PSUM tile, so
  one exp covers the pair and s1 PE time halves vs per-head issue.
- Attention runs as ONE flat software pipeline over 16 blocks of (head-pair x
  query-quarter) x 16 key chunks; att@V trails the exp by 2 chunks and flows
  across block boundaries with no drain.
- Projections are split into ~2-matmul microtasks and dripped into the kc
  loops (1 pull/kc) in data-arrival order; a filler feeding block N is always
  fully emitted before block N's first reader (Tile deps only point backward).
- DMA is descriptor-row-bound (~25ns/row), so the hot transfers use fat rows:
  the host pre-packs weights as [128, ic*512] (8KB rows) and xv as eight
  512KB t-chunk-major blocks (4KB rows, each holding all 8 contraction slices
  for 2 key chunks).  Queues: sync = wv + xv blocks (v chase), scalar = wk +
  xk rows (k chase), gpsimd = wq + biases + xq rows + Wo.

fp16 is used for the whole q/k/scores path: bf16's 8-bit mantissa gives score
errors ~0.05 which exp() amplifies to ~2e-2 output error; fp16 keeps it ~3e-3.
"""

from collections import deque
from contextlib import ExitStack

import numpy as np

import concourse.bacc as bacc
import concourse.mybir as mybir
import concourse.tile as tile
from concourse.bass_utils import run_bass_kernel_spmd

F32 = mybir.dt.float32
F16 = mybir.dt.float16
BF16 = mybir.dt.bfloat16

D = 1024          # d_model
HD = 64           # head dim
NH_CORE = 8       # heads per core
OC = NH_CORE * HD # per-core q/k/v output dims (512)
N_CORES = 8
NI = D // 128     # contraction chunks for projections
NOC = OC // 128   # o-chunks (head pairs)
NDC = D // 128    # output-dim chunks for the final projection
VG = 256          # xv host block width (2 key chunks)


def build_kernel(S=2048):
    nc = bacc.Bacc("TRN2", target_bir_lowering=False, debug=False)

    NVG = S // VG             # 8 xv blocks

    xq_d = nc.dram_tensor("xqT", (D, S), F16, kind="ExternalInput")
    xk_d = nc.dram_tensor("xkT", (D, S), F16, kind="ExternalInput")
    xv_d = nc.dram_tensor("xvB", (NVG, 128, NI * VG), F16, kind="ExternalInput")
    wq_d = nc.dram_tensor("WqP", (128, NI * OC), F16, kind="ExternalInput")
    wk_d = nc.dram_tensor("WkP", (128, NI * OC), F16, kind="ExternalInput")
    wv_d = nc.dram_tensor("WvP", (128, NI * OC), F16, kind="ExternalInput")
    wo_d = nc.dram_tensor("WoP", (128, NOC * D), F16, kind="ExternalInput")
    bq_d = nc.dram_tensor("bq", (OC,), F32, kind="ExternalInput")
    bk_d = nc.dram_tensor("bk", (OC,), F32, kind="ExternalInput")
    y_d = nc.dram_tensor("yT", (D, S), BF16, kind="ExternalOutput")

    NKC = S // 128            # key chunks (16)
    QQ = 512                  # query-quarter size (s1 moving width per head)
    NQQ = S // QQ             # 4

    Exp = mybir.ActivationFunctionType.Exp
    Mult = mybir.AluOpType.mult

    with tile.TileContext(nc) as tc, ExitStack() as ctx:
        wpool = ctx.enter_context(tc.tile_pool(name="w", bufs=1))
        xpool = ctx.enter_context(tc.tile_pool(name="x", bufs=1))
        spool = ctx.enter_context(tc.tile_pool(name="seq", bufs=1))
        epool = ctx.enter_context(tc.tile_pool(name="e", bufs=3))
        evpool = ctx.enter_context(tc.tile_pool(name="ev", bufs=2))
        npool = ctx.enter_context(tc.tile_pool(name="nrm", bufs=2))
        pjpool = ctx.enter_context(tc.tile_pool(name="pj", bufs=2, space="PSUM"))
        s1pool = ctx.enter_context(tc.tile_pool(name="s1", bufs=2, space="PSUM"))
        accpool = ctx.enter_context(tc.tile_pool(name="acc", bufs=2, space="PSUM"))

        # ---- resident weights / biases / x slabs ----
        wq_sb = wpool.tile([128, NI, OC], F16, tag="wq")
        wk_sb = wpool.tile([128, NI, OC], F16, tag="wk")
        wv_sb = wpool.tile([128, NI, OC], F16, tag="wv")
        wo_sb = wpool.tile([128, NOC, D], F16, tag="wo")
        bq_sb = wpool.tile([128, NOC], F32, tag="bq")
        bk_sb = wpool.tile([128, NOC], F32, tag="bk")

        xq_sb = xpool.tile([128, NI, S], F16, tag="xq")
        xk_sb = xpool.tile([128, NI, S], F16, tag="xk")
        xvg_sb = [xpool.tile([128, NI, VG], F16, tag="xvg", bufs=4,
                             name=f"xvg{g}") for g in range(NVG)]

        # queue loads balanced by measured rates (sync ~70GB/s is slowest):
        # sync: wv + xv g0-3; scalar: wk + xk + xv g4-5; gpsimd: wq + biases
        # + xq + xv g6-7 + wo.  Within each queue, need-order.
        def xv_dma(engine, g):
            engine.dma_start(out=xvg_sb[g],
                             in_=xv_d.ap()[g].rearrange("p (ic t) -> p ic t", ic=NI))

        nc.sync.dma_start(out=wv_sb,
                          in_=wv_d.ap().rearrange("p (ic o) -> p ic o", ic=NI))
        for g in range(4):
            xv_dma(nc.sync, g)
        nc.scalar.dma_start(out=wk_sb,
                            in_=wk_d.ap().rearrange("p (ic o) -> p ic o", ic=NI))
        for ic in range(NI):
            nc.scalar.dma_start(out=xk_sb[:, ic, :],
                                in_=xk_d.ap()[ic * 128:(ic + 1) * 128, :])
        xv_dma(nc.scalar, 4)
        xv_dma(nc.scalar, 5)
        nc.gpsimd.dma_start(out=wq_sb,
                            in_=wq_d.ap().rearrange("p (ic o) -> p ic o", ic=NI))
        nc.gpsimd.dma_start(out=bq_sb, in_=bq_d.ap().rearrange("(c p) -> p c", p=128))
        nc.gpsimd.dma_start(out=bk_sb, in_=bk_d.ap().rearrange("(c p) -> p c", p=128))
        for ic in range(NI):
            nc.gpsimd.dma_start(out=xq_sb[:, ic, :],
                                in_=xq_d.ap()[ic * 128:(ic + 1) * 128, :])
        xv_dma(nc.gpsimd, 6)
        xv_dma(nc.gpsimd, 7)
        nc.gpsimd.dma_start(out=wo_sb,
                            in_=wo_d.ap().rearrange("p (oc d) -> p oc d", oc=NOC))

        # ---- per-sequence slabs ----
        v_sb = spool.tile([128, NKC, NH_CORE * (HD + 1)], BF16, tag="v")
        qT_sb = spool.tile([128, NOC, S], F16, tag="qT")
        kT_sb = spool.tile([128, NOC, S], F16, tag="kT")
        att_sb = spool.tile([128, NOC, S], BF16, tag="att")

        # ---- microtask unit factories (units are atomic: each holds its pj
        # ---- ring slots until its trailing DVE op) ----
        def v_sub(g):
            """v projection for one xv block (2 key chunks): 16 MMs + copy."""
            state = {}
            def mk_mm(ic):
                def f():
                    if "a" not in state:
                        state["a"] = pjpool.tile([128, OC], F32, tag="pj",
                                                 name=f"vpsa{g}")
                        state["b"] = pjpool.tile([128, OC], F32, tag="pj",
                                                 name=f"vpsb{g}")
                    for j, key in enumerate(("a", "b")):
                        nc.tensor.matmul(
                            state[key][:, :],
                            xvg_sb[g][:, ic, j * 128:(j + 1) * 128],
                            wv_sb[:, ic, :],
                            start=(ic == 0), stop=(ic == NI - 1),
                        )
                return f
            def fin():
                for j, key in enumerate(("a", "b")):
                    vv = v_sb[:, 2 * g + j, :].rearrange("p (h c) -> p h c",
                                                         h=NH_CORE)
                    nc.vector.tensor_copy(
                        out=vv[:, :, 0:HD],
                        in_=state[key][:, :].rearrange("p (h c) -> p h c", c=HD),
                    )
                    nc.vector.memset(vv[:, :, HD:HD + 1], 1.0)
            return [mk_mm(ic) for ic in range(NI)] + [fin]

        def proj_pair(w_sb, x_sb, b_sb, dst, hp, tqa, tqb, label):
            """q/k projection for (head-pair, two 512-col chunks): each
            stationary w[ic] feeds both moving chunks."""
            state = {}
            def mk_mm(ic):
                def f():
                    if "a" not in state:
                        state["a"] = pjpool.tile([128, QQ], F32, tag="pj",
                                                 name=f"pj{label}{hp}_{tqa}")
                        state["b"] = pjpool.tile([128, QQ], F32, tag="pj",
                                                 name=f"pj{label}{hp}_{tqb}")
                    for key, tq in (("a", tqa), ("b", tqb)):
                        nc.tensor.matmul(
                            state[key][:, :],
                            w_sb[:, ic, hp * 128:(hp + 1) * 128],
                            x_sb[:, ic, tq * QQ:(tq + 1) * QQ],
                            start=(ic == 0), stop=(ic == NI - 1),
                        )
                return f
            def bias():
                for key, tq in (("a", tqa), ("b", tqb)):
                    nc.vector.tensor_scalar_add(
                        out=dst[:, hp, tq * QQ:(tq + 1) * QQ],
                        in0=state[key][:, :],
                        scalar1=b_sb[:, hp:hp + 1],
                    )
            return [mk_mm(ic) for ic in range(NI)] + [bias]

        def y_unit(qq, dc):
            """output projection for one (128 d rows, 512 queries):
            4 accum matmuls as 2 microtasks + bf16 store."""
            state = {}
            def mk_mm(ocp):
                def f():
                    if "pj" not in state:
                        state["pj"] = pjpool.tile([128, QQ], F32, tag="pj",
                                                  name=f"yps{qq}_{dc}")
                    for oc in (2 * ocp, 2 * ocp + 1):
                        nc.tensor.matmul(
                            state["pj"][:, :],
                            wo_sb[:, oc, dc * 128:(dc + 1) * 128],
                            att_sb[:, oc, qq * QQ:(qq + 1) * QQ],
                            start=(oc == 0), stop=(oc == NOC - 1),
                        )
                return f
            def store():
                y_sb = evpool.tile([128, QQ], BF16, tag="yev",
                                   name=f"yev{qq}_{dc}")
                nc.vector.tensor_copy(out=y_sb[:, :], in_=state["pj"][:, :])
                nc.sync.dma_start(
                    out=y_d.ap()[dc * 128:(dc + 1) * 128,
                                 qq * QQ:(qq + 1) * QQ],
                    in_=y_sb[:, :],
                )
            return [mk_mm(0), mk_mm(1), store]

        # ---- filler schedule ----
        fillers = deque()

        def pull(n=1):
            for _ in range(n):
                if fillers:
                    fillers.popleft()()

        def run_unit(u):
            for f in u:
                f()

        # ---- prologue: v proj chasing the xv blocks, hp0 k/q interleaved
        # ---- in arrival order.  Units are atomic (pj slot discipline).
        v_sub_units = [v_sub(g) for g in range(NVG)]
        run_unit(v_sub_units[0])
        run_unit(v_sub_units[1])
        run_unit(v_sub_units[2])
        run_unit(proj_pair(wk_sb, xk_sb, bk_sb, kT_sb, 0, 0, 1, "k"))
        run_unit(v_sub_units[3])
        run_unit(proj_pair(wk_sb, xk_sb, bk_sb, kT_sb, 0, 2, 3, "k"))
        run_unit(v_sub_units[4])
        run_unit(v_sub_units[5])
        run_unit(proj_pair(wq_sb, xq_sb, bq_sb, qT_sb, 0, 0, 1, "q"))
        run_unit(v_sub_units[6])
        run_unit(v_sub_units[7])

        # ---- filler population in need-order ----
        # Block (hp,0) starts at pull 32*hp; each pair's k+q(0,1) units (27
        # microtasks) sit within the preceding 32 pulls.
        for hp in (1, 2, 3):
            fillers.extend(proj_pair(wk_sb, xk_sb, bk_sb, kT_sb, hp, 0, 1, "k"))
            fillers.extend(proj_pair(wk_sb, xk_sb, bk_sb, kT_sb, hp, 2, 3, "k"))
            fillers.extend(proj_pair(wq_sb, xq_sb, bq_sb, qT_sb, hp, 0, 1, "q"))
        # hp0's remaining q quarters are not needed until block 8 (pull 128)
        fillers.extend(proj_pair(wq_sb, xq_sb, bq_sb, qT_sb, 0, 2, 3, "q"))

        later_q = []
        for hp in (1, 2, 3):
            later_q.append(proj_pair(wq_sb, xq_sb, bq_sb, qT_sb, hp, 2, 3, "q"))

        # block order: quarters 0/1 paired per head-pair (gives the k/q
        # prefetch two blocks of lead per pair), then quarters 2/3 swept
        # by-quarter so y(qq2) can run as filler and only y(qq3) tails
        order = ([(hp, qq) for hp in range(NOC) for qq in (0, 1)]
                 + [(hp, 2) for hp in range(NOC)]
                 + [(hp, 3) for hp in range(NOC)])

        # ---- flat attention pipeline ----
        pend = deque()
        accs_by_bi = {}
        qq_done = {qq: 0 for qq in range(NQQ)}

        def finalize(bi, hp, qq):
            # copy accs to SBUF first: frees the acc PSUM ring slots early so
            # the next block's s2 never stalls on them
            qoff = qq * QQ
            accs = accs_by_bi.pop(bi)
            asbs = []
            for hl in range(2):
                asb = npool.tile([65, QQ], F32, tag="accsb", bufs=2,
                                 name=f"asb{bi}_{hl}")
                nc.vector.tensor_copy(out=asb[:, :], in_=accs[hl][0:65, :])
                asbs.append(asb)
            for hl in range(2):
                off = hl * 64
                asb = asbs[hl]
                rt = npool.tile([1, QQ], F32, tag="rtmp", bufs=2,
                                name=f"rt{bi}_{hl}")
                nc.vector.tensor_copy(out=rt[:, :], in_=asb[64:65, :])
                nc.vector.reciprocal_approx_fast(out=rt[:, :], in_=rt[:, :])
                bc = npool.tile([64, QQ], F32, tag="bcast", bufs=2,
                                name=f"bc{bi}_{hl}")
                nc.gpsimd.partition_broadcast(out_ap=bc[:, :], in_ap=rt[:, :])
                nc.vector.tensor_tensor(
                    out=att_sb[off:off + 64, hp, qoff:qoff + QQ],
                    in0=asb[0:64, :],
                    in1=bc[:, :],
                    op=Mult,
                )
            qq_done[qq] += 1
            if qq_done[qq] == NOC and qq < 3:
                for dc in range(NDC):
                    fillers.extend(y_unit(qq, dc))
            # qq3's y is emitted in the tail (acc ring, hp3-oc last) so most
            # of it overlaps the final block's normalize

        def s2_pop():
            bi2, hp2, qq2, e2, kc2 = pend.popleft()
            if kc2 == 0:
                accs_by_bi[bi2] = [
                    accpool.tile([128, QQ], F32, tag="acc", name=f"acc{bi2}_{hl}")
                    for hl in range(2)
                ]
            accs = accs_by_bi[bi2]
            for hl in range(2):
                h = 2 * hp2 + hl
                nc.tensor.matmul(
                    accs[hl][0:65, :],
                    v_sb[:, kc2, h * (HD + 1):(h + 1) * (HD + 1)],
                    e2[:, hl * QQ:(hl + 1) * QQ],
                    start=(kc2 == 0), stop=(kc2 == NKC - 1),
                )
            if kc2 == NKC - 1:
                finalize(bi2, hp2, qq2)

        for bi, (hp, qq) in enumerate(order):
            if bi == 4:
                for u in later_q:
                    fillers.extend(u)
            qoff = qq * QQ
            for kc in range(NKC):
                s1 = s1pool.tile([128, 2 * QQ], F32, tag="s1",
                                 name=f"s1_{bi}_{kc}")
                # the two heads' score MMs land on row groups 0/64 and
                # stream concurrently; one exp covers the fused tile
                for hl in range(2):
                    off = hl * 64
                    nc.tensor.matmul(
                        s1[:, hl * QQ:(hl + 1) * QQ],
                        kT_sb[off:off + 64, hp, kc * 128:(kc + 1) * 128],
                        qT_sb[off:off + 64, hp, qoff:qoff + QQ],
                        start=True, stop=True,
                    )
                e = epool.tile([128, 2 * QQ], BF16, tag="e", name=f"e{bi}_{kc}")
                nc.scalar.activation(out=e[:, :], in_=s1[:, :], func=Exp)
                pend.append((bi, hp, qq, e, kc))
                if len(pend) > 2:
                    s2_pop()
                pull(1)
        while pend:
            s2_pop()

        # tail: y(qq3) through the freed acc PSUM ring (2-deep pipelining);
        # oc0-2 matmuls run during the final normalize (their att quarters
        # are long ready), only the hp3-dependent oc3 waits for it
        for dc in range(NDC):
            state = {}
            yp = accpool.tile([128, QQ], F32, tag="acc", name=f"yt{dc}")
            for ocs in ((0, 1), (2,), (3,)):
                for oc in ocs:
                    nc.tensor.matmul(
                        yp[:, :],
                        wo_sb[:, oc, dc * 128:(dc + 1) * 128],
                        att_sb[:, oc, 3 * QQ:4 * QQ],
                        start=(oc == 0), stop=(oc == NOC - 1),
                    )
            y_sb = evpool.tile([128, QQ], BF16, tag="yev", name=f"yevt{dc}")
            nc.vector.tensor_copy(out=y_sb[:, :], in_=yp[:, :])
            nc.sync.dma_start(
                out=y_d.ap()[dc * 128:(dc + 1) * 128, 3 * QQ:4 * QQ],
                in_=y_sb[:, :],
            )
        while fillers:
            fillers.popleft()()

    nc.compile()
    return nc


def make_in_maps(query, key, value, Wq, bq, Wk, bk, Wv, bv, Wo, bo):
    """Shard + lay out full inputs for the 8 cores: core = 2*n + g.

    Weights are host-packed to [128, ic*512] (8KB DMA rows) and xv to
    t-chunk-major 512KB blocks (4KB rows) — DMA is descriptor-row-bound."""
    f16 = np.float16
    N, S, _ = query.shape
    NVG = S // VG

    def pack_w(WT):  # [D, oc] -> [128, ic*oc]
        ni, oc = WT.shape[0] // 128, WT.shape[1]
        return np.ascontiguousarray(
            WT.reshape(ni, 128, oc).transpose(1, 0, 2).reshape(128, ni * oc)
        ).astype(f16)

    per_g = {}
    for g in range(2):
        osl = slice(g * OC, (g + 1) * OC)
        per_g[g] = dict(
            WqP=pack_w(Wq[osl, :].T),
            WkP=pack_w(Wk[osl, :].T),
            WvP=pack_w(Wv[osl, :].T),
            WoP=pack_w(Wo[:, osl].T),
            bq=np.ascontiguousarray(bq[osl]).astype(np.float32),
            bk=np.ascontiguousarray(bk[osl]).astype(np.float32),
        )
    in_maps = []
    for n in range(N):
        xqT = np.ascontiguousarray(query[n].T).astype(f16)
        xkT = np.ascontiguousarray(key[n].T).astype(f16)
        xvT = value[n].T  # [D, S]
        xvB = np.ascontiguousarray(
            xvT.reshape(NI, 128, NVG, VG).transpose(2, 1, 0, 3)
               .reshape(NVG, 128, NI * VG)
        ).astype(f16)
        for g in range(2):
            m = dict(xqT=xqT, xkT=xkT, xvB=xvB)
            m.update(per_g[g])
            in_maps.append(m)
    return in_maps


_BUILT = None


def _get_built():
    global _BUILT
    if _BUILT is None:
        _BUILT = build_kernel(2048)
    return _BUILT


def kernel(query, key, value, Wq, bq, Wk, bk, Wv, bv, Wo, bo, _results=None):
    query = np.asarray(query, np.float32)
    key = np.asarray(key, np.float32)
    value = np.asarray(value, np.float32)
    Wq, bq = np.asarray(Wq, np.float32), np.asarray(bq, np.float32)
    Wk, bk = np.asarray(Wk, np.float32), np.asarray(bk, np.float32)
    Wv, bv = np.asarray(Wv, np.float32), np.asarray(bv, np.float32)
    Wo, bo = np.asarray(Wo, np.float32), np.asarray(bo, np.float32)

    N, S, _ = query.shape
    if _results is None:
        nc = _get_built()
        in_maps = make_in_maps(query, key, value, Wq, bq, Wk, bk, Wv, bv, Wo, bo)
        res = run_bass_kernel_spmd(nc, in_maps, list(range(N_CORES)))
        _results = res.results

    const = bv @ Wo.T + bo  # host-folded bias terms
    out = np.empty((N, S, D), np.float32)
    for n in range(N):
        yT = (_results[2 * n]["yT"].astype(np.float32)
              + _results[2 * n + 1]["yT"].astype(np.float32))
        out[n] = yT.T + const
    return out
